# revision 4
# baseline (speedup 1.0000x reference)
"""Trainium2 Bass kernel for Grossberg dynamics, v2.

dS/dt = (-DECAY*s + (B-s)*relu(exc) - (C+s)*relu(inh)) / TAU, masked on actions.

Data-parallel over 8 cores; per core 32768 agents, 16 macros x 2048 agents.
Agent a = m*2048 + p*16 + g (partition-major).

Host layout: W interleaved per agent: wbuf[p][(g,t,i,j)] (t = pos/neg), fp16.
Multiply (DVE 2x): prod[p][g,t,i,j] = wbuf * s_h[p][g,j] (broadcast over t,i
via merged stride-0 dims). Reduce over j: 2x tensor_tensor tree:
  L1[0:8] = prod[0:8] + prod[9:17]   (j=8 leftover)
  L2[0:4] = L1[0:4] + L1[4:8]
  L3[0:2] = L2[0:2] + L2[2:4]
  mv = L3[0] + L3[1] (1x) ; mv += prod[8] (1x)
Epilogue on ACT (sigmoid/relu) + Pool (adds/mults) only; DVE does reciprocal.
"""

import numpy as np

import concourse.bass as bass
import concourse.bacc as bacc
import concourse.mybir as mybir
from concourse.tile import TileContext
from concourse.bass_utils import run_bass_kernel_spmd

P = 128
N = 17
NN = N * N
NCORES = 8
B_TOTAL = 262144
B_CORE = B_TOTAL // NCORES  # 32768
G = 16                      # agents per partition per macro-tile
MACROS = B_CORE // (P * G)  # 16

FP = mybir.dt.float32
FH = mybir.dt.float16
AX = mybir.AxisListType
OP = mybir.AluOpType
AF = mybir.ActivationFunctionType

TAU, DECAY, B_CAP, C_FLOOR = 0.8, 0.15, 1.0, 0.1
LAT_INHIB, DIV_SIGMA = 3.0, 0.3
ALPHA, BETA = 1.5, 0.75
INV_TAU = 1.0 / TAU
U_BIAS = DECAY * INV_TAU          # dS = R_e - 0.1*R_i - s*(U_BIAS + R_e + R_i)
LAT_DEN_C = DIV_SIGMA + 1e-6


def build_program():
    nc = bacc.Bacc()
    _cb = nc.alloc_sbuf_tensor(f"const-float32-{U_BIAS}", [128, 1], FP)
    nc.gpsimd.memset(_cb.ap(), U_BIAS)
    nc.const_aps.aps[(FP, U_BIAS)] = _cb.ap()
    _cu = nc.alloc_sbuf_tensor("const-ubias-h", [128, 1], FH)
    nc.gpsimd.memset(_cu.ap(), U_BIAS)
    # wq[a][(t,i,j)] interleaved pos/neg per agent, fp16
    wq_d = nc.dram_tensor("wq", [B_CORE, 2 * NN], FH, kind="ExternalInput")
    pk_d = nc.dram_tensor("pk", [B_CORE, 38], FH, kind="ExternalInput")
    out_d = nc.dram_tensor("out", [B_CORE, N], FH, kind="ExternalOutput")

    wq_v = wq_d[:, :].rearrange("(m p g) q -> m p (g q)", p=P, g=G)
    pk_v = pk_d[:, :].rearrange("(m p g) x -> m p (g x)", p=P, g=G)
    out_v = out_d[:, :].rearrange("(m p g) n -> m p (g n)", p=P, g=G)

    GN = G * N
    W2 = 2 * G * NN             # 9248 cols of weights per partition
    with TileContext(nc) as tc:
        with (
            tc.tile_pool(name="big2", bufs=2) as pool2,
            tc.tile_pool(name="big1", bufs=1) as pool1,
        ):
            for m in range(MACROS):
                # ---- loads ----
                wbuf = pool2.tile([P, W2], FH, tag="wbuf")
                nc.sync.dma_start(out=wbuf[:, 0 : W2 // 2], in_=wq_v[m][:, 0 : W2 // 2])
                nc.sync.dma_start(out=wbuf[:, W2 // 2 :], in_=wq_v[m][:, W2 // 2 :])
                sh = pool2.tile([P, GN], FH, tag="sh")
                nc.sync.dma_start(out=sh[:], in_=sh_v[m])
                pert = pool2.tile([P, GN], FH, tag="pert")
                nc.sync.dma_start(out=pert[:], in_=pt_v[m])
                feas = pool2.tile([P, G * 4], FH, tag="feas")
                nc.sync.dma_start(out=feas[:], in_=fs_v[m])

                pk3 = pkt.rearrange("p (g x) -> p g x", x=38)
                s3 = pk3[:, :, 0:N]
                pt3 = pk3[:, :, N : 2 * N]
                fs3 = pk3[:, :, 2 * N : 38]

                # ---- Pool: state-only lateral prep + valence (early) ----
                ve = pool2.tile([P, 4 * G], FH, tag="ve")
                ve3 = ve.rearrange("p (g f) -> p g f", f=4)
                nc.gpsimd.tensor_tensor(
                    out=ve3, in0=s3[:, :, 13:17], in1=pt3[:, :, 13:17], op=OP.add
                )
                a01 = pool2.tile([P, 2 * G], FH, tag="a01")
                a013 = a01.rearrange("p (g f) -> p g f", f=2)
                nc.gpsimd.tensor_tensor(
                    out=a013, in0=s3[:, :, 9:11], in1=s3[:, :, 11:13], op=OP.add
                )
                suma = pool2.tile([P, G], FH, tag="suma")
                nc.gpsimd.tensor_tensor(
                    out=suma[:, :, None], in0=a013[:, :, 0:1], in1=a013[:, :, 1:2], op=OP.add
                )
                other = pool2.tile([P, 4 * G], FH, tag="other")
                other3 = other.rearrange("p (g f) -> p g f", f=4)
                nc.gpsimd.tensor_tensor(
                    out=other3,
                    in0=suma[:, :, None].broadcast_to([P, G, 4]),
                    in1=s3[:, :, 9:13],
                    op=OP.subtract,
                )

                # ---- ACT: gates + env relus (early) ----
                gates = pool2.tile([P, 8 * G], FH, tag="gates")
                g4 = gates.rearrange("p (g t f) -> p g t f", t=2, f=4)
                nc.scalar.activation(g4[:, :, 0, :], ve3, AF.Sigmoid, scale=ALPHA)
                nc.scalar.activation(g4[:, :, 1, :], ve3, AF.Sigmoid, scale=-BETA)
                env = pool2.tile([P, 18 * G], FH, tag="env")
                e4 = env.rearrange("p (g t f) -> p g t f", t=2, f=9)
                nc.scalar.activation(e4[:, :, 0, :], pt3[:, :, 0:9], AF.Relu)
                nc.scalar.activation(e4[:, :, 1, :], pt3[:, :, 0:9], AF.Relu, scale=-1.0)

                # ---- DVE: multiply + tree reduce ----
                prod = pool2.tile([P, W2], FH, tag="prod")
                w5 = wbuf.rearrange("p (g q j) -> p g q j", q=2 * N, j=N)
                p5 = prod.rearrange("p (g q j) -> p g q j", q=2 * N, j=N)
                s5 = s3[:, :, None, :].broadcast_to([P, G, 2 * N, N])
                if m == 0:
                    # split first multiply so it starts after the first DMA half
                    H2 = W2 // 2
                    GH = G // 2
                    w5a = wbuf[:, 0:H2].rearrange("p (g q j) -> p g q j", q=2 * N, j=N)
                    p5a = prod[:, 0:H2].rearrange("p (g q j) -> p g q j", q=2 * N, j=N)
                    s5a = s3[:, 0:GH, None, :].broadcast_to([P, GH, 2 * N, N])
                    nc.vector.tensor_tensor(out=p5a, in0=w5a, in1=s5a, op=OP.mult)
                    w5b = wbuf[:, H2:].rearrange("p (g q j) -> p g q j", q=2 * N, j=N)
                    p5b = prod[:, H2:].rearrange("p (g q j) -> p g q j", q=2 * N, j=N)
                    s5b = s3[:, GH:G, None, :].broadcast_to([P, GH, 2 * N, N])
                    nc.vector.tensor_tensor(out=p5b, in0=w5b, in1=s5b, op=OP.mult)
                else:
                    nc.vector.tensor_tensor(out=p5, in0=w5, in1=s5, op=OP.mult)
                K = 2 * G * N           # 544 segments
                pk = prod.rearrange("p (k j) -> p k j", j=N)
                l1 = pool2.tile([P, K * 8], FH, tag="l1")
                l1k = l1.rearrange("p (k j) -> p k j", j=8)
                nc.vector.tensor_tensor(
                    out=l1k, in0=pk[:, :, 0:8], in1=pk[:, :, 9:17], op=OP.add
                )
                l2 = pool2.tile([P, K * 4], FH, tag="l2")
                l2k = l2.rearrange("p (k j) -> p k j", j=4)
                nc.vector.tensor_tensor(
                    out=l2k, in0=l1k[:, :, 0:4], in1=l1k[:, :, 4:8], op=OP.add
                )
                l3 = pool2.tile([P, K * 2], FH, tag="l3")
                l3k = l3.rearrange("p (k j) -> p k j", j=2)
                nc.vector.tensor_tensor(
                    out=l3k, in0=l2k[:, :, 0:2], in1=l2k[:, :, 2:4], op=OP.add
                )
                # DVE: lateral den/recip (dep: other, ready early)
                den = pool2.tile([P, 4 * G], FP, tag="den")
                nc.vector.tensor_scalar(
                    out=den[:], in0=other[:],
                    scalar1=1.0 / LAT_INHIB, scalar2=LAT_DEN_C / LAT_INHIB,
                    op0=OP.mult, op1=OP.add,
                )
                recip = pool2.tile([P, 4 * G], FP, tag="recip")
                nc.vector.reciprocal(recip[:], den[:])

                # ---- Pool: finish reduce (L4 + leftover j=8) ----
                mv = pool2.tile([P, K], FH, tag="mv")
                nc.gpsimd.tensor_tensor(
                    out=mv[:, :, None], in0=l3k[:, :, 0:1], in1=l3k[:, :, 1:2], op=OP.add
                )
                nc.gpsimd.tensor_tensor(
                    out=mv[:, :, None], in0=mv[:, :, None], in1=pk[:, :, 8:9], op=OP.add
                )
                mv4 = mv.rearrange("p (g t n) -> p g t n", t=2, n=N)

                # ---- Pool: gate, env, lateral application ----
                nc.gpsimd.tensor_tensor(
                    out=mv4[:, :, :, 9:13], in0=mv4[:, :, :, 9:13], in1=g4, op=OP.mult
                )
                nc.gpsimd.tensor_tensor(
                    out=mv4[:, :, :, 0:9], in0=mv4[:, :, :, 0:9], in1=e4, op=OP.add
                )
                lat = pool2.tile([P, 4 * G], FH, tag="lat")
                nc.gpsimd.tensor_tensor(
                    out=lat[:], in0=other[:], in1=recip[:], op=OP.mult
                )
                lat3 = lat.rearrange("p (g f) -> p g f", f=4)
                nc.gpsimd.tensor_tensor(
                    out=mv4[:, :, 1, 9:13], in0=mv4[:, :, 1, 9:13], in1=lat3, op=OP.add
                )

                # ---- combine: dS = R_e - 0.1*R_i - s*(U_BIAS + R_e + R_i) ----
                r = pool2.tile([P, K], FH, tag="r")
                nc.scalar.activation(r[:], mv[:], AF.Relu, scale=INV_TAU)
                r4 = r.rearrange("p (g t n) -> p g t n", t=2, n=N)
                t1 = pool1.tile([P, GN], FH, tag="t1")
                t13 = t1.rearrange("p (g n) -> p g n", n=N)
                nc.gpsimd.tensor_tensor(
                    out=t13, in0=r4[:, :, 0, :], in1=r4[:, :, 1, :], op=OP.add
                )
                v0 = pool1.tile([P, GN], FH, tag="v0")
                v03 = v0.rearrange("p (g n) -> p g n", n=N)
                nc.scalar.activation(v03, r4[:, :, 1, :], AF.Copy, scale=-C_FLOOR)
                t2 = pool1.tile([P, GN], FH, tag="t2")
                nc.gpsimd.tensor_tensor(
                    out=t2[:], in0=t1[:], in1=_cu.ap()[:, 0:1].broadcast_to([P, GN]),
                    op=OP.add,
                )
                u = pool1.tile([P, GN], FH, tag="u")
                nc.gpsimd.tensor_tensor(out=u[:], in0=t2[:], in1=sh[:], op=OP.mult)
                v = pool1.tile([P, GN], FH, tag="v")
                v3 = v.rearrange("p (g n) -> p g n", n=N)
                nc.gpsimd.tensor_tensor(out=v3, in0=r4[:, :, 0, :], in1=v03, op=OP.add)
                ob = pool2.tile([P, GN], FH, tag="ob")
                nc.gpsimd.tensor_tensor(out=ob[:], in0=v[:], in1=u[:], op=OP.subtract)
                ob3 = ob.rearrange("p (g n) -> p g n", n=N)
                fs3 = feas.rearrange("p (g f) -> p g f", f=4)
                nc.gpsimd.tensor_tensor(
                    out=ob3[:, :, 9:13], in0=ob3[:, :, 9:13], in1=fs3, op=OP.mult
                )

                nc.scalar.dma_start(out=out_v[m], in_=ob[:])
    if not nc.is_finalized():
        nc.finalize()
    return nc


def make_in_maps(state, w_pos, w_neg, feasibility, perturbation):
    state = np.asarray(state, dtype=np.float32)
    w_pos = np.asarray(w_pos, dtype=np.float32)
    w_neg = np.asarray(w_neg, dtype=np.float32)
    feas = np.asarray(feasibility, dtype=np.float32)
    pert = np.asarray(perturbation, dtype=np.float32)

    # wq[a][(t,i,j)]
    wq = np.stack([w_pos, w_neg], axis=1).reshape(B_TOTAL, 2 * NN).astype(np.float16)
    pk = np.concatenate([state, pert, feas], axis=1).astype(np.float16)  # [a][38]
    in_maps = []
    for c in range(NCORES):
        sl = slice(c * B_CORE, (c + 1) * B_CORE)
        in_maps.append(
            {
                "wq": np.ascontiguousarray(wq[sl]),
                "pk": np.ascontiguousarray(pk[sl]),
            }
        )
    return in_maps


def gather(results):
    return np.concatenate([r["out"] for r in results], axis=0).astype(np.float32)


def kernel(t=None, state=None, W_pos=None, W_neg=None, feasibility=None, perturbation=None, **_):
    nc = build_program()
    in_maps = make_in_maps(state, W_pos, W_neg, feasibility, perturbation)
    res = run_bass_kernel_spmd(nc, in_maps, list(range(NCORES)))
    return gather(res.results)


if __name__ == "__main__":
    rng = np.random.default_rng(0)
    inputs = {
        "t": rng.standard_normal(1).astype(np.float32),
        "state": rng.random((B_TOTAL, N), dtype=np.float32),
        "W_pos": rng.random((B_TOTAL, N, N), dtype=np.float32),
        "W_neg": rng.random((B_TOTAL, N, N), dtype=np.float32),
        "feasibility": rng.random((B_TOTAL, 4), dtype=np.float32),
        "perturbation": rng.standard_normal((B_TOTAL, N)).astype(np.float32),
    }
    out = kernel(**inputs)
    print(out.shape, out.dtype)


# revision 5
# speedup vs baseline: 1.1169x; 1.1169x over previous
"""Trainium2 Bass kernel for Grossberg dynamics, v2.

dS/dt = (-DECAY*s + (B-s)*relu(exc) - (C+s)*relu(inh)) / TAU, masked on actions.

Data-parallel over 8 cores; per core 32768 agents, 16 macros x 2048 agents.
Agent a = m*2048 + p*16 + g (partition-major).

Host layout: W interleaved per agent: wbuf[p][(g,t,i,j)] (t = pos/neg), fp16.
Multiply (DVE 2x): prod[p][g,t,i,j] = wbuf * s_h[p][g,j] (broadcast over t,i
via merged stride-0 dims). Reduce over j: 2x tensor_tensor tree:
  L1[0:8] = prod[0:8] + prod[9:17]   (j=8 leftover)
  L2[0:4] = L1[0:4] + L1[4:8]
  L3[0:2] = L2[0:2] + L2[2:4]
  mv = L3[0] + L3[1] (1x) ; mv += prod[8] (1x)
Epilogue on ACT (sigmoid/relu) + Pool (adds/mults) only; DVE does reciprocal.
"""

import numpy as np

import concourse.bass as bass
import concourse.bacc as bacc
import concourse.mybir as mybir
from concourse.tile import TileContext
from concourse.bass_utils import run_bass_kernel_spmd

P = 128
N = 17
NN = N * N
NCORES = 8
B_TOTAL = 262144
B_CORE = B_TOTAL // NCORES  # 32768
G = 16                      # agents per partition per macro-tile
MACROS = B_CORE // (P * G)  # 16

FP = mybir.dt.float32
FH = mybir.dt.float16
AX = mybir.AxisListType
OP = mybir.AluOpType
AF = mybir.ActivationFunctionType

TAU, DECAY, B_CAP, C_FLOOR = 0.8, 0.15, 1.0, 0.1
LAT_INHIB, DIV_SIGMA = 3.0, 0.3
ALPHA, BETA = 1.5, 0.75
INV_TAU = 1.0 / TAU
U_BIAS = DECAY * INV_TAU          # dS = R_e - 0.1*R_i - s*(U_BIAS + R_e + R_i)
LAT_DEN_C = DIV_SIGMA + 1e-6


def build_program():
    nc = bacc.Bacc()
    _cb = nc.alloc_sbuf_tensor(f"const-float32-{U_BIAS}", [128, 1], FP)
    nc.gpsimd.memset(_cb.ap(), U_BIAS)
    nc.const_aps.aps[(FP, U_BIAS)] = _cb.ap()
    _cu = nc.alloc_sbuf_tensor("const-ubias-h", [128, 1], FH)
    nc.gpsimd.memset(_cu.ap(), U_BIAS)
    # wq[a][(t,i,j)] interleaved pos/neg per agent, fp16
    wq_d = nc.dram_tensor("wq", [B_CORE, 2 * NN], FH, kind="ExternalInput")
    pk_d = nc.dram_tensor("pk", [B_CORE, 38], FH, kind="ExternalInput")
    out_d = nc.dram_tensor("out", [B_CORE, N], FH, kind="ExternalOutput")

    wq_v = wq_d[:, :].rearrange("(m p g) q -> m p (g q)", p=P, g=G)
    pk_v = pk_d[:, :].rearrange("(m p g) x -> m p (g x)", p=P, g=G)
    out_v = out_d[:, :].rearrange("(m p g) n -> m p (g n)", p=P, g=G)

    GN = G * N
    W2 = 2 * G * NN             # 9248 cols of weights per partition
    with TileContext(nc) as tc:
        with (
            tc.tile_pool(name="big2", bufs=2) as pool2,
            tc.tile_pool(name="big1", bufs=1) as pool1,
        ):
            for m in range(MACROS):
                # ---- loads ----
                wbuf = pool2.tile([P, W2], FH, tag="wbuf")
                nc.sync.dma_start(out=wbuf[:, 0 : W2 // 2], in_=wq_v[m][:, 0 : W2 // 2])
                nc.sync.dma_start(out=wbuf[:, W2 // 2 :], in_=wq_v[m][:, W2 // 2 :])
                sh = pool2.tile([P, GN], FH, tag="sh")
                nc.sync.dma_start(out=sh[:], in_=sh_v[m])
                pert = pool2.tile([P, GN], FH, tag="pert")
                nc.sync.dma_start(out=pert[:], in_=pt_v[m])
                feas = pool2.tile([P, G * 4], FH, tag="feas")
                nc.sync.dma_start(out=feas[:], in_=fs_v[m])

                pk3 = pkt.rearrange("p (g x) -> p g x", x=38)
                s3 = pk3[:, :, 0:N]
                pt3 = pk3[:, :, N : 2 * N]
                fs3 = pk3[:, :, 2 * N : 38]

                # ---- Pool: state-only lateral prep + valence (early) ----
                ve = pool2.tile([P, 4 * G], FH, tag="ve")
                ve3 = ve.rearrange("p (g f) -> p g f", f=4)
                nc.gpsimd.tensor_tensor(
                    out=ve3, in0=s3[:, :, 13:17], in1=pt3[:, :, 13:17], op=OP.add
                )
                a01 = pool2.tile([P, 2 * G], FH, tag="a01")
                a013 = a01.rearrange("p (g f) -> p g f", f=2)
                nc.gpsimd.tensor_tensor(
                    out=a013, in0=s3[:, :, 9:11], in1=s3[:, :, 11:13], op=OP.add
                )
                suma = pool2.tile([P, G], FH, tag="suma")
                nc.gpsimd.tensor_tensor(
                    out=suma[:, :, None], in0=a013[:, :, 0:1], in1=a013[:, :, 1:2], op=OP.add
                )
                other = pool2.tile([P, 4 * G], FH, tag="other")
                other3 = other.rearrange("p (g f) -> p g f", f=4)
                nc.gpsimd.tensor_tensor(
                    out=other3,
                    in0=suma[:, :, None].broadcast_to([P, G, 4]),
                    in1=s3[:, :, 9:13],
                    op=OP.subtract,
                )

                # ---- ACT: gates + env relus (early) ----
                gates = pool2.tile([P, 8 * G], FH, tag="gates")
                g4 = gates.rearrange("p (g t f) -> p g t f", t=2, f=4)
                nc.scalar.activation(g4[:, :, 0, :], ve3, AF.Sigmoid, scale=ALPHA)
                nc.scalar.activation(g4[:, :, 1, :], ve3, AF.Sigmoid, scale=-BETA)
                env = pool2.tile([P, 18 * G], FH, tag="env")
                e4 = env.rearrange("p (g t f) -> p g t f", t=2, f=9)
                nc.scalar.activation(e4[:, :, 0, :], pt3[:, :, 0:9], AF.Relu)
                nc.scalar.activation(e4[:, :, 1, :], pt3[:, :, 0:9], AF.Relu, scale=-1.0)

                # ---- DVE: multiply + tree reduce ----
                prod = pool2.tile([P, W2], FH, tag="prod")
                w5 = wbuf.rearrange("p (g q j) -> p g q j", q=2 * N, j=N)
                p5 = prod.rearrange("p (g q j) -> p g q j", q=2 * N, j=N)
                s5 = s3[:, :, None, :].broadcast_to([P, G, 2 * N, N])
                nsplit = 4 if m == 0 else (2 if m == 1 else 1)
                if nsplit > 1:
                    # split early multiplies so each starts as its W slice lands
                    GH = G // nsplit
                    HB = W2 // nsplit
                    for q in range(nsplit):
                        w5q = wbuf[:, q * HB : (q + 1) * HB].rearrange(
                            "p (g q j) -> p g q j", q=2 * N, j=N
                        )
                        p5q = prod[:, q * HB : (q + 1) * HB].rearrange(
                            "p (g q j) -> p g q j", q=2 * N, j=N
                        )
                        s5q = s3[:, q * GH : (q + 1) * GH, None, :].broadcast_to(
                            [P, GH, 2 * N, N]
                        )
                        nc.vector.tensor_tensor(out=p5q, in0=w5q, in1=s5q, op=OP.mult)
                else:
                    nc.vector.tensor_tensor(out=p5, in0=w5, in1=s5, op=OP.mult)
                K = 2 * G * N           # 544 segments
                pk = prod.rearrange("p (k j) -> p k j", j=N)
                l1 = pool2.tile([P, K * 8], FH, tag="l1")
                l1k = l1.rearrange("p (k j) -> p k j", j=8)
                nc.vector.tensor_tensor(
                    out=l1k, in0=pk[:, :, 0:8], in1=pk[:, :, 9:17], op=OP.add
                )
                l2 = pool2.tile([P, K * 4], FH, tag="l2")
                l2k = l2.rearrange("p (k j) -> p k j", j=4)
                nc.vector.tensor_tensor(
                    out=l2k, in0=l1k[:, :, 0:4], in1=l1k[:, :, 4:8], op=OP.add
                )
                l3 = pool2.tile([P, K * 2], FH, tag="l3")
                l3k = l3.rearrange("p (k j) -> p k j", j=2)
                nc.vector.tensor_tensor(
                    out=l3k, in0=l2k[:, :, 0:2], in1=l2k[:, :, 2:4], op=OP.add
                )
                # DVE: lateral den/recip (dep: other, ready early)
                den = pool2.tile([P, 4 * G], FP, tag="den")
                nc.vector.tensor_scalar(
                    out=den[:], in0=other[:],
                    scalar1=1.0 / LAT_INHIB, scalar2=LAT_DEN_C / LAT_INHIB,
                    op0=OP.mult, op1=OP.add,
                )
                recip = pool2.tile([P, 4 * G], FP, tag="recip")
                nc.vector.reciprocal(recip[:], den[:])

                # ---- Pool: finish reduce (L4 + leftover j=8) ----
                mv = pool2.tile([P, K], FH, tag="mv")
                nc.gpsimd.tensor_tensor(
                    out=mv[:, :, None], in0=l3k[:, :, 0:1], in1=l3k[:, :, 1:2], op=OP.add
                )
                nc.gpsimd.tensor_tensor(
                    out=mv[:, :, None], in0=mv[:, :, None], in1=pk[:, :, 8:9], op=OP.add
                )
                mv4 = mv.rearrange("p (g t n) -> p g t n", t=2, n=N)

                # ---- Pool: gate, env, lateral application ----
                nc.gpsimd.tensor_tensor(
                    out=mv4[:, :, :, 9:13], in0=mv4[:, :, :, 9:13], in1=g4, op=OP.mult
                )
                nc.gpsimd.tensor_tensor(
                    out=mv4[:, :, :, 0:9], in0=mv4[:, :, :, 0:9], in1=e4, op=OP.add
                )
                lat = pool2.tile([P, 4 * G], FH, tag="lat")
                nc.gpsimd.tensor_tensor(
                    out=lat[:], in0=other[:], in1=recip[:], op=OP.mult
                )
                lat3 = lat.rearrange("p (g f) -> p g f", f=4)
                nc.gpsimd.tensor_tensor(
                    out=mv4[:, :, 1, 9:13], in0=mv4[:, :, 1, 9:13], in1=lat3, op=OP.add
                )

                # ---- combine: dS = R_e - 0.1*R_i - s*(U_BIAS + R_e + R_i) ----
                r = pool2.tile([P, K], FH, tag="r")
                nc.scalar.activation(r[:], mv[:], AF.Relu, scale=INV_TAU)
                r4 = r.rearrange("p (g t n) -> p g t n", t=2, n=N)
                t1 = pool1.tile([P, GN], FH, tag="t1")
                t13 = t1.rearrange("p (g n) -> p g n", n=N)
                nc.gpsimd.tensor_tensor(
                    out=t13, in0=r4[:, :, 0, :], in1=r4[:, :, 1, :], op=OP.add
                )
                v0 = pool1.tile([P, GN], FH, tag="v0")
                v03 = v0.rearrange("p (g n) -> p g n", n=N)
                nc.scalar.activation(v03, r4[:, :, 1, :], AF.Copy, scale=-C_FLOOR)
                t2 = pool1.tile([P, GN], FH, tag="t2")
                nc.gpsimd.tensor_tensor(
                    out=t2[:], in0=t1[:], in1=_cu.ap()[:, 0:1].broadcast_to([P, GN]),
                    op=OP.add,
                )
                u = pool1.tile([P, GN], FH, tag="u")
                nc.gpsimd.tensor_tensor(out=u[:], in0=t2[:], in1=sh[:], op=OP.mult)
                v = pool1.tile([P, GN], FH, tag="v")
                v3 = v.rearrange("p (g n) -> p g n", n=N)
                nc.gpsimd.tensor_tensor(out=v3, in0=r4[:, :, 0, :], in1=v03, op=OP.add)
                ob = pool2.tile([P, GN], FH, tag="ob")
                nc.gpsimd.tensor_tensor(out=ob[:], in0=v[:], in1=u[:], op=OP.subtract)
                ob3 = ob.rearrange("p (g n) -> p g n", n=N)
                fs3 = feas.rearrange("p (g f) -> p g f", f=4)
                nc.gpsimd.tensor_tensor(
                    out=ob3[:, :, 9:13], in0=ob3[:, :, 9:13], in1=fs3, op=OP.mult
                )

                nc.scalar.dma_start(out=out_v[m], in_=ob[:])
    if not nc.is_finalized():
        nc.finalize()
    return nc


def make_in_maps(state, w_pos, w_neg, feasibility, perturbation):
    state = np.asarray(state, dtype=np.float32)
    w_pos = np.asarray(w_pos, dtype=np.float32)
    w_neg = np.asarray(w_neg, dtype=np.float32)
    feas = np.asarray(feasibility, dtype=np.float32)
    pert = np.asarray(perturbation, dtype=np.float32)

    # wq[a][(t,i,j)]
    wq = np.stack([w_pos, w_neg], axis=1).reshape(B_TOTAL, 2 * NN).astype(np.float16)
    pk = np.concatenate([state, pert, feas], axis=1).astype(np.float16)  # [a][38]
    in_maps = []
    for c in range(NCORES):
        sl = slice(c * B_CORE, (c + 1) * B_CORE)
        in_maps.append(
            {
                "wq": np.ascontiguousarray(wq[sl]),
                "pk": np.ascontiguousarray(pk[sl]),
            }
        )
    return in_maps


def gather(results):
    return np.concatenate([r["out"] for r in results], axis=0).astype(np.float32)


def kernel(t=None, state=None, W_pos=None, W_neg=None, feasibility=None, perturbation=None, **_):
    nc = build_program()
    in_maps = make_in_maps(state, W_pos, W_neg, feasibility, perturbation)
    res = run_bass_kernel_spmd(nc, in_maps, list(range(NCORES)))
    return gather(res.results)


if __name__ == "__main__":
    rng = np.random.default_rng(0)
    inputs = {
        "t": rng.standard_normal(1).astype(np.float32),
        "state": rng.random((B_TOTAL, N), dtype=np.float32),
        "W_pos": rng.random((B_TOTAL, N, N), dtype=np.float32),
        "W_neg": rng.random((B_TOTAL, N, N), dtype=np.float32),
        "feasibility": rng.random((B_TOTAL, 4), dtype=np.float32),
        "perturbation": rng.standard_normal((B_TOTAL, N)).astype(np.float32),
    }
    out = kernel(**inputs)
    print(out.shape, out.dtype)


# revision 6
# speedup vs baseline: 1.1193x; 1.0021x over previous
"""Trainium2 Bass kernel for Grossberg dynamics, v2.

dS/dt = (-DECAY*s + (B-s)*relu(exc) - (C+s)*relu(inh)) / TAU, masked on actions.

Data-parallel over 8 cores; per core 32768 agents, 16 macros x 2048 agents.
Agent a = m*2048 + p*16 + g (partition-major).

Host layout: W interleaved per agent: wbuf[p][(g,t,i,j)] (t = pos/neg), fp16.
Multiply (DVE 2x): prod[p][g,t,i,j] = wbuf * s_h[p][g,j] (broadcast over t,i
via merged stride-0 dims). Reduce over j: 2x tensor_tensor tree:
  L1[0:8] = prod[0:8] + prod[9:17]   (j=8 leftover)
  L2[0:4] = L1[0:4] + L1[4:8]
  L3[0:2] = L2[0:2] + L2[2:4]
  mv = L3[0] + L3[1] (1x) ; mv += prod[8] (1x)
Epilogue on ACT (sigmoid/relu) + Pool (adds/mults) only; DVE does reciprocal.
"""

import numpy as np

import concourse.bass as bass
import concourse.bacc as bacc
import concourse.mybir as mybir
from concourse.tile import TileContext
from concourse.bass_utils import run_bass_kernel_spmd

P = 128
N = 17
NN = N * N
NCORES = 8
B_TOTAL = 262144
B_CORE = B_TOTAL // NCORES  # 32768
G = 16                      # agents per partition per macro-tile
MACROS = B_CORE // (P * G)  # 16

FP = mybir.dt.float32
FH = mybir.dt.float16
AX = mybir.AxisListType
OP = mybir.AluOpType
AF = mybir.ActivationFunctionType

TAU, DECAY, B_CAP, C_FLOOR = 0.8, 0.15, 1.0, 0.1
LAT_INHIB, DIV_SIGMA = 3.0, 0.3
ALPHA, BETA = 1.5, 0.75
INV_TAU = 1.0 / TAU
U_BIAS = DECAY * INV_TAU          # dS = R_e - 0.1*R_i - s*(U_BIAS + R_e + R_i)
LAT_DEN_C = DIV_SIGMA + 1e-6


def build_program():
    nc = bacc.Bacc()
    _cb = nc.alloc_sbuf_tensor(f"const-float32-{U_BIAS}", [128, 1], FP)
    nc.gpsimd.memset(_cb.ap(), U_BIAS)
    nc.const_aps.aps[(FP, U_BIAS)] = _cb.ap()
    _cu = nc.alloc_sbuf_tensor("const-ubias-h", [128, 1], FH)
    nc.gpsimd.memset(_cu.ap(), U_BIAS)
    # wq[a][(t,i,j)] interleaved pos/neg per agent, fp16
    wq_d = nc.dram_tensor("wq", [B_CORE, 2 * NN], FH, kind="ExternalInput")
    pk_d = nc.dram_tensor("pk", [B_CORE, 38], FH, kind="ExternalInput")
    out_d = nc.dram_tensor("out", [B_CORE, N], FH, kind="ExternalOutput")

    wq_v = wq_d[:, :].rearrange("(m p g) q -> m p (g q)", p=P, g=G)
    pk_v = pk_d[:, :].rearrange("(m p g) x -> m p (g x)", p=P, g=G)
    out_v = out_d[:, :].rearrange("(m p g) n -> m p (g n)", p=P, g=G)

    GN = G * N
    W2 = 2 * G * NN             # 9248 cols of weights per partition
    with TileContext(nc) as tc:
        with (
            tc.tile_pool(name="big2", bufs=2) as pool2,
            tc.tile_pool(name="big1", bufs=1) as pool1,
        ):
            for m in range(MACROS):
                # ---- loads ----
                wbuf = pool2.tile([P, W2], FH, tag="wbuf")
                nc.sync.dma_start(out=wbuf[:, 0 : W2 // 2], in_=wq_v[m][:, 0 : W2 // 2])
                nc.sync.dma_start(out=wbuf[:, W2 // 2 :], in_=wq_v[m][:, W2 // 2 :])
                sh = pool2.tile([P, GN], FH, tag="sh")
                nc.sync.dma_start(out=sh[:], in_=sh_v[m])
                pert = pool2.tile([P, GN], FH, tag="pert")
                nc.sync.dma_start(out=pert[:], in_=pt_v[m])
                feas = pool2.tile([P, G * 4], FH, tag="feas")
                nc.sync.dma_start(out=feas[:], in_=fs_v[m])

                pk3 = pkt.rearrange("p (g x) -> p g x", x=38)
                s3 = pk3[:, :, 0:N]
                pt3 = pk3[:, :, N : 2 * N]
                fs3 = pk3[:, :, 2 * N : 38]

                # ---- Pool: state-only lateral prep + valence (early) ----
                ve = pool2.tile([P, 4 * G], FH, tag="ve")
                ve3 = ve.rearrange("p (g f) -> p g f", f=4)
                nc.gpsimd.tensor_tensor(
                    out=ve3, in0=s3[:, :, 13:17], in1=pt3[:, :, 13:17], op=OP.add
                )
                a01 = pool2.tile([P, 2 * G], FH, tag="a01")
                a013 = a01.rearrange("p (g f) -> p g f", f=2)
                nc.gpsimd.tensor_tensor(
                    out=a013, in0=s3[:, :, 9:11], in1=s3[:, :, 11:13], op=OP.add
                )
                suma = pool2.tile([P, G], FH, tag="suma")
                nc.gpsimd.tensor_tensor(
                    out=suma[:, :, None], in0=a013[:, :, 0:1], in1=a013[:, :, 1:2], op=OP.add
                )
                other = pool2.tile([P, 4 * G], FH, tag="other")
                other3 = other.rearrange("p (g f) -> p g f", f=4)
                nc.gpsimd.tensor_tensor(
                    out=other3,
                    in0=suma[:, :, None].broadcast_to([P, G, 4]),
                    in1=s3[:, :, 9:13],
                    op=OP.subtract,
                )

                # ---- ACT: gates + env relus (early) ----
                gates = pool2.tile([P, 8 * G], FH, tag="gates")
                g4 = gates.rearrange("p (g t f) -> p g t f", t=2, f=4)
                nc.scalar.activation(g4[:, :, 0, :], ve3, AF.Sigmoid, scale=ALPHA)
                nc.scalar.activation(g4[:, :, 1, :], ve3, AF.Sigmoid, scale=-BETA)
                env = pool2.tile([P, 18 * G], FH, tag="env")
                e4 = env.rearrange("p (g t f) -> p g t f", t=2, f=9)
                nc.scalar.activation(e4[:, :, 0, :], pt3[:, :, 0:9], AF.Relu)
                nc.scalar.activation(e4[:, :, 1, :], pt3[:, :, 0:9], AF.Relu, scale=-1.0)

                # ---- DVE: multiply + tree reduce ----
                prod = pool2.tile([P, W2], FH, tag="prod")
                w5 = wbuf.rearrange("p (g q j) -> p g q j", q=2 * N, j=N)
                p5 = prod.rearrange("p (g q j) -> p g q j", q=2 * N, j=N)
                s5 = s3[:, :, None, :].broadcast_to([P, G, 2 * N, N])
                nsplit = 2 if m == 0 else 1
                if nsplit > 1:
                    # split early multiplies so each starts as its W slice lands
                    GH = G // nsplit
                    HB = W2 // nsplit
                    for q in range(nsplit):
                        w5q = wbuf[:, q * HB : (q + 1) * HB].rearrange(
                            "p (g q j) -> p g q j", q=2 * N, j=N
                        )
                        p5q = prod[:, q * HB : (q + 1) * HB].rearrange(
                            "p (g q j) -> p g q j", q=2 * N, j=N
                        )
                        s5q = s3[:, q * GH : (q + 1) * GH, None, :].broadcast_to(
                            [P, GH, 2 * N, N]
                        )
                        nc.vector.tensor_tensor(out=p5q, in0=w5q, in1=s5q, op=OP.mult)
                else:
                    nc.vector.tensor_tensor(out=p5, in0=w5, in1=s5, op=OP.mult)
                K = 2 * G * N           # 544 segments
                pk = prod.rearrange("p (k j) -> p k j", j=N)
                l1 = pool2.tile([P, K * 8], FH, tag="l1")
                l1k = l1.rearrange("p (k j) -> p k j", j=8)
                nc.vector.tensor_tensor(
                    out=l1k, in0=pk[:, :, 0:8], in1=pk[:, :, 9:17], op=OP.add
                )
                l2 = pool2.tile([P, K * 4], FH, tag="l2")
                l2k = l2.rearrange("p (k j) -> p k j", j=4)
                nc.vector.tensor_tensor(
                    out=l2k, in0=l1k[:, :, 0:4], in1=l1k[:, :, 4:8], op=OP.add
                )
                l3 = pool2.tile([P, K * 2], FH, tag="l3")
                l3k = l3.rearrange("p (k j) -> p k j", j=2)
                nc.vector.tensor_tensor(
                    out=l3k, in0=l2k[:, :, 0:2], in1=l2k[:, :, 2:4], op=OP.add
                )
                # DVE: lateral den/recip (dep: other, ready early)
                den = pool2.tile([P, 4 * G], FP, tag="den")
                nc.vector.tensor_scalar(
                    out=den[:], in0=other[:],
                    scalar1=1.0 / LAT_INHIB, scalar2=LAT_DEN_C / LAT_INHIB,
                    op0=OP.mult, op1=OP.add,
                )
                recip = pool2.tile([P, 4 * G], FP, tag="recip")
                nc.vector.reciprocal(recip[:], den[:])

                # ---- Pool: finish reduce (L4 + leftover j=8) ----
                mv = pool2.tile([P, K], FH, tag="mv")
                nc.gpsimd.tensor_tensor(
                    out=mv[:, :, None], in0=l3k[:, :, 0:1], in1=l3k[:, :, 1:2], op=OP.add
                )
                nc.gpsimd.tensor_tensor(
                    out=mv[:, :, None], in0=mv[:, :, None], in1=pk[:, :, 8:9], op=OP.add
                )
                mv4 = mv.rearrange("p (g t n) -> p g t n", t=2, n=N)

                # ---- Pool: gate, env, lateral application ----
                nc.gpsimd.tensor_tensor(
                    out=mv4[:, :, :, 9:13], in0=mv4[:, :, :, 9:13], in1=g4, op=OP.mult
                )
                nc.gpsimd.tensor_tensor(
                    out=mv4[:, :, :, 0:9], in0=mv4[:, :, :, 0:9], in1=e4, op=OP.add
                )
                lat = pool2.tile([P, 4 * G], FH, tag="lat")
                nc.gpsimd.tensor_tensor(
                    out=lat[:], in0=other[:], in1=recip[:], op=OP.mult
                )
                lat3 = lat.rearrange("p (g f) -> p g f", f=4)
                nc.gpsimd.tensor_tensor(
                    out=mv4[:, :, 1, 9:13], in0=mv4[:, :, 1, 9:13], in1=lat3, op=OP.add
                )

                # ---- combine: dS = R_e - 0.1*R_i - s*(U_BIAS + R_e + R_i) ----
                r = pool2.tile([P, K], FH, tag="r")
                nc.scalar.activation(r[:], mv[:], AF.Relu, scale=INV_TAU)
                r4 = r.rearrange("p (g t n) -> p g t n", t=2, n=N)
                t1 = pool1.tile([P, GN], FH, tag="t1")
                t13 = t1.rearrange("p (g n) -> p g n", n=N)
                nc.gpsimd.tensor_tensor(
                    out=t13, in0=r4[:, :, 0, :], in1=r4[:, :, 1, :], op=OP.add
                )
                v0 = pool1.tile([P, GN], FH, tag="v0")
                v03 = v0.rearrange("p (g n) -> p g n", n=N)
                nc.scalar.activation(v03, r4[:, :, 1, :], AF.Copy, scale=-C_FLOOR)
                t2 = pool1.tile([P, GN], FH, tag="t2")
                nc.gpsimd.tensor_tensor(
                    out=t2[:], in0=t1[:], in1=_cu.ap()[:, 0:1].broadcast_to([P, GN]),
                    op=OP.add,
                )
                u = pool1.tile([P, GN], FH, tag="u")
                nc.gpsimd.tensor_tensor(out=u[:], in0=t2[:], in1=sh[:], op=OP.mult)
                v = pool1.tile([P, GN], FH, tag="v")
                v3 = v.rearrange("p (g n) -> p g n", n=N)
                nc.gpsimd.tensor_tensor(out=v3, in0=r4[:, :, 0, :], in1=v03, op=OP.add)
                ob = pool2.tile([P, GN], FH, tag="ob")
                nc.gpsimd.tensor_tensor(out=ob[:], in0=v[:], in1=u[:], op=OP.subtract)
                ob3 = ob.rearrange("p (g n) -> p g n", n=N)
                fs3 = feas.rearrange("p (g f) -> p g f", f=4)
                nc.gpsimd.tensor_tensor(
                    out=ob3[:, :, 9:13], in0=ob3[:, :, 9:13], in1=fs3, op=OP.mult
                )

                nc.scalar.dma_start(out=out_v[m], in_=ob[:])
    if not nc.is_finalized():
        nc.finalize()
    return nc


def make_in_maps(state, w_pos, w_neg, feasibility, perturbation):
    state = np.asarray(state, dtype=np.float32)
    w_pos = np.asarray(w_pos, dtype=np.float32)
    w_neg = np.asarray(w_neg, dtype=np.float32)
    feas = np.asarray(feasibility, dtype=np.float32)
    pert = np.asarray(perturbation, dtype=np.float32)

    # wq[a][(t,i,j)]
    wq = np.stack([w_pos, w_neg], axis=1).reshape(B_TOTAL, 2 * NN).astype(np.float16)
    pk = np.concatenate([state, pert, feas], axis=1).astype(np.float16)  # [a][38]
    in_maps = []
    for c in range(NCORES):
        sl = slice(c * B_CORE, (c + 1) * B_CORE)
        in_maps.append(
            {
                "wq": np.ascontiguousarray(wq[sl]),
                "pk": np.ascontiguousarray(pk[sl]),
            }
        )
    return in_maps


def gather(results):
    return np.concatenate([r["out"] for r in results], axis=0).astype(np.float32)


def kernel(t=None, state=None, W_pos=None, W_neg=None, feasibility=None, perturbation=None, **_):
    nc = build_program()
    in_maps = make_in_maps(state, W_pos, W_neg, feasibility, perturbation)
    res = run_bass_kernel_spmd(nc, in_maps, list(range(NCORES)))
    return gather(res.results)


if __name__ == "__main__":
    rng = np.random.default_rng(0)
    inputs = {
        "t": rng.standard_normal(1).astype(np.float32),
        "state": rng.random((B_TOTAL, N), dtype=np.float32),
        "W_pos": rng.random((B_TOTAL, N, N), dtype=np.float32),
        "W_neg": rng.random((B_TOTAL, N, N), dtype=np.float32),
        "feasibility": rng.random((B_TOTAL, 4), dtype=np.float32),
        "perturbation": rng.standard_normal((B_TOTAL, N)).astype(np.float32),
    }
    out = kernel(**inputs)
    print(out.shape, out.dtype)
